# revision 3
# baseline (speedup 1.0000x reference)
"""ContextKnowledgeEncoder kernel for Trainium2 (8 NeuronCores, data-parallel over N).

Per batch row n (one per core):
  - gather knowledge-token embeddings (32 cand x 128 tok) via dma_gather  [8 MiB]
  - gather context-token embeddings (256 tok)                             [0.5 MiB]
  - masked-sum pooling via PE matmuls (block-diagonal mask, PSUM accum)
  - ck_attn = know_use . context_use, masked with ck_mask
  - argmax -> candidate selection -> re-gather chosen sentence -> outputs
"""
import numpy as np
from contextlib import ExitStack

N, K, Tk, Tc, V, D = 8, 32, 128, 256, 32000, 512
NEGINF = -1e20
NCHUNK = 4           # know gather/pool chunks (divisor of K)
POOL_DT = "f32"      # "f32" (exact-ish) or "f32r" (fast fp32 PE mode)

_compiled = None


def _build():
    import concourse.bacc as bacc
    import concourse.bass as bass
    import concourse.mybir as mybir
    import concourse.tile as tile
    from concourse.library_config import mlp

    f32 = mybir.dt.float32
    i32 = mybir.dt.int32
    i16 = mybir.dt.int16
    u8 = mybir.dt.uint8
    Alu = mybir.AluOpType
    Act = mybir.ActivationFunctionType
    AX = mybir.AxisListType.X

    nc = bacc.Bacc("TRN2", debug=False)

    emb = nc.dram_tensor("emb", [V, D], f32, kind="ExternalInput")
    kidx16 = nc.dram_tensor("kidx16", [128, K * Tk // 16], i16, kind="ExternalInput")
    sidx16 = nc.dram_tensor("sidx16", [128, Tc // 16], i16, kind="ExternalInput")
    ktokt = nc.dram_tensor("ktokt", [128, K], i32, kind="ExternalInput")      # [t, k]
    stokt = nc.dram_tensor("stokt", [128, 2], i32, kind="ExternalInput")      # [t%128, t//128]
    stokrow = nc.dram_tensor("stokrow", [1, Tc], f32, kind="ExternalInput")   # src tokens row
    cconsts = nc.dram_tensor("cconsts", [K, 2], f32, kind="ExternalInput")    # ck_f | ck_neg
    rconsts = nc.dram_tensor("rconsts", [1, 66], f32, kind="ExternalInput")   # iota|oh_host|use

    full_enc = nc.dram_tensor("full_enc", [Tk + Tc, D], f32, kind="ExternalOutput")
    full_mask = nc.dram_tensor("full_mask", [Tk + Tc], u8, kind="ExternalOutput")
    ck_attn = nc.dram_tensor("ck_attn", [K], f32, kind="ExternalOutput")

    CHK = K // NCHUNK  # candidates per chunk

    with tile.TileContext(nc) as tc, ExitStack() as ctx:
        sb = ctx.enter_context(tc.tile_pool(name="sb", bufs=1))
        ps = ctx.enter_context(tc.tile_pool(name="ps", bufs=1, space="PSUM"))

        nc.gpsimd.load_library(mlp)

        # ---- small input loads
        kidx_sb = sb.tile([128, K * Tk // 16], i16)
        nc.sync.dma_start(out=kidx_sb[:], in_=kidx16[:, :])
        sidx_sb = sb.tile([128, Tc // 16], i16)
        nc.sync.dma_start(out=sidx_sb[:], in_=sidx16[:, :])
        ktokt_sb = sb.tile([128, K], i32)
        nc.sync.dma_start(out=ktokt_sb[:], in_=ktokt[:, :])
        stokt_sb = sb.tile([128, 2], i32)
        nc.sync.dma_start(out=stokt_sb[:], in_=stokt[:, :])
        stokrow_sb = sb.tile([1, Tc], f32)
        nc.sync.dma_start(out=stokrow_sb[:], in_=stokrow[:, :])
        cc_sb = sb.tile([K, 2], f32)
        nc.sync.dma_start(out=cc_sb[:], in_=cconsts[:, :])
        rc_sb = sb.tile([1, 66], f32)
        nc.sync.dma_start(out=rc_sb[:], in_=rconsts[:, :])
        iota_row = rc_sb[0:1, 0:K]
        oh_host = rc_sb[0:1, K:2 * K]
        use_s = rc_sb[0:1, 64:65]

        # ---- gathers (ctx first: its pooled vector is needed before attn)
        ctx_g = sb.tile([128, 2 * D], f32)
        nc.gpsimd.dma_gather(ctx_g[:].rearrange("p (s d) -> p s d", s=2),
                             emb[:, :], sidx_sb[:], Tc, Tc, D)
        know_g = sb.tile([128, K * D], f32)
        know_g3 = know_g[:].rearrange("p (k d) -> p k d", k=K)
        for c in range(NCHUNK):
            nidx = CHK * Tk
            nc.gpsimd.dma_gather(know_g3[:, c * CHK:(c + 1) * CHK, :], emb[:, :],
                                 kidx_sb[:, c * nidx // 16:(c + 1) * nidx // 16],
                                 nidx, nidx, D)

        # ---- masks, counts, scales (independent of gathers)
        kmaskT = sb.tile([128, K], f32)
        nc.vector.tensor_scalar(out=kmaskT[:], in0=ktokt_sb[:], scalar1=0,
                                scalar2=None, op0=Alu.not_equal)
        ktokt_f = sb.tile([128, K], f32)
        nc.vector.tensor_copy(out=ktokt_f[:], in_=ktokt_sb[:])
        smask = sb.tile([128, 2], f32)
        nc.vector.tensor_scalar(out=smask[:], in0=stokt_sb[:], scalar1=0,
                                scalar2=None, op0=Alu.not_equal)
        ones_col = sb.tile([128, 1], f32)
        nc.vector.memset(ones_col[:], 1.0)
        ones_row = sb.tile([1, 128], f32)
        nc.vector.memset(ones_row[:], 1.0)

        # Z: block-diagonal mask for pooling accumulation. Z[:, k*K + k] = kmaskT[:, k]
        Z = sb.tile([128, K * K], f32)
        nc.vector.memset(Z[:], 0.0)
        zap = Z[:, :]
        zdiag = bass.AP(tensor=zap.tensor, offset=zap.offset,
                        ap=[zap.ap[0], [K + 1, K]])
        nc.vector.tensor_copy(out=zdiag, in_=kmaskT[:, :])

        # counts via PE (exact): kcount[k] = sum_t kmaskT[t,k]
        kcount_ps = ps.tile([K, 1], f32)
        nc.tensor.matmul(out=kcount_ps[:], lhsT=kmaskT[:], rhs=ones_col[:],
                         start=True, stop=True)
        kscale = sb.tile([K, 1], f32)
        nc.scalar.activation(out=kscale[:], in_=kcount_ps[:], func=Act.Sqrt,
                             scale=float(D))
        nc.vector.reciprocal(out=kscale[:], in_=kscale[:])

        scount = sb.tile([1, 1], f32)
        smask_row = sb.tile([1, Tc], f32)
        nc.vector.tensor_scalar(out=smask_row[:], in0=stokrow_sb[:], scalar1=0,
                                scalar2=None, op0=Alu.not_equal)
        nc.vector.tensor_reduce(out=scount[:], in_=smask_row[:], axis=AX, op=Alu.add)
        sscale = sb.tile([1, 1], f32)
        nc.scalar.activation(out=sscale[:], in_=scount[:], func=Act.Sqrt,
                             scale=float(D))
        nc.vector.reciprocal(out=sscale[:], in_=sscale[:])

        # ---- ctx pooling -> pctx (scaled)
        pctx_ps = ps.tile([1, D], f32)
        for s in range(2):
            nc.tensor.matmul(out=pctx_ps[:], lhsT=smask[:, s:s + 1],
                             rhs=ctx_g[:, s * D:(s + 1) * D],
                             start=(s == 0), stop=(s == 1))
        pctx = sb.tile([1, D], f32)
        nc.vector.tensor_scalar(out=pctx[:], in0=pctx_ps[:], scalar1=sscale[:],
                                scalar2=None, op0=Alu.mult)

        # broadcast pctx to K partitions via PE
        bc_ps = ps.tile([K, D], f32)
        nc.tensor.matmul(out=bc_ps[:], lhsT=ones_row[0:1, 0:K], rhs=pctx[:],
                         start=True, stop=True)

        # ---- know pooling (chunked, accumulating into one PSUM tile)
        pk_ps = ps.tile([K, D], f32)
        if POOL_DT == "f32r":
            import concourse.mybir as _m
            zz = Z[:, :].bitcast(_m.dt.float32r)
            kg = know_g[:].bitcast(_m.dt.float32r)
        for k in range(K):
            if POOL_DT == "f32r":
                lhs = zz[:, k * K:(k + 1) * K]
                rhs = kg[:, k * D:(k + 1) * D]
            else:
                lhs = Z[:, k * K:(k + 1) * K]
                rhs = know_g[:, k * D:(k + 1) * D]
            nc.tensor.matmul(out=pk_ps[:], lhsT=lhs, rhs=rhs,
                             start=(k == 0), stop=(k == K - 1))
        know_use = sb.tile([K, D], f32)
        nc.vector.tensor_scalar(out=know_use[:], in0=pk_ps[:], scalar1=kscale[:],
                                scalar2=None, op0=Alu.mult)

        # ---- attn = rowwise dot(know_use, pctx) ; mask with ck
        prod = sb.tile([K, D], f32)
        nc.vector.tensor_tensor(out=prod[:], in0=know_use[:], in1=bc_ps[:], op=Alu.mult)
        attn = sb.tile([K, 1], f32)
        nc.vector.tensor_reduce(out=attn[:], in_=prod[:], axis=AX, op=Alu.add)
        attn_m = sb.tile([K, 1], f32)
        nc.vector.scalar_tensor_tensor(out=attn_m[:], in0=attn[:],
                                       scalar=cc_sb[:, 0:1], in1=cc_sb[:, 1:2],
                                       op0=Alu.mult, op1=Alu.add)

        # ---- transpose attn to a row, write ck_attn
        sq = sb.tile([32, 32], f32)
        nc.vector.memset(sq[:], 0.0)
        nc.vector.tensor_copy(out=sq[:, 0:1], in_=attn_m[:])
        sqT = sb.tile([32, 32], f32)
        nc.vector.transpose(out=sqT[:], in_=sq[:])
        attn_row = sqT[0:1, 0:K]
        nc.sync.dma_start(out=ck_attn[:], in_=attn_row)

        # ---- argmax (first occurrence) via max + reversed-iota trick
        maxv = sb.tile([1, 1], f32)
        nc.vector.tensor_reduce(out=maxv[:], in_=attn_row, axis=AX, op=Alu.max)
        eq = sb.tile([1, K], f32)
        nc.vector.tensor_scalar(out=eq[:], in0=attn_row, scalar1=maxv[:],
                                scalar2=None, op0=Alu.is_equal)
        t1 = sb.tile([1, K], f32)
        # iota_row holds (64 - k); eq*(64-k) -> max = 64 - argmax
        nc.vector.tensor_tensor(out=t1[:], in0=eq[:], in1=iota_row, op=Alu.mult)
        m2 = sb.tile([1, 1], f32)
        nc.vector.tensor_reduce(out=m2[:], in_=t1[:], axis=AX, op=Alu.max)
        cs_id = sb.tile([1, 1], f32)
        nc.vector.tensor_scalar(out=cs_id[:], in0=m2[:], scalar1=-1.0, scalar2=64.0,
                                op0=Alu.mult, op1=Alu.add)

        # onehot row; blend with host-provided onehot (use_cs_ids path)
        ohd = sb.tile([1, K], f32)
        nc.vector.tensor_scalar(out=ohd[:], in0=iota_row, scalar1=-1.0, scalar2=64.0,
                                op0=Alu.mult, op1=Alu.add)  # = k
        nc.vector.tensor_scalar(out=ohd[:], in0=ohd[:], scalar1=cs_id[:],
                                scalar2=None, op0=Alu.is_equal)
        d1 = sb.tile([1, K], f32)
        nc.vector.tensor_tensor(out=d1[:], in0=oh_host, in1=ohd[:], op=Alu.subtract)
        oh_row = sb.tile([1, K], f32)
        nc.vector.scalar_tensor_tensor(out=oh_row[:], in0=d1[:], scalar=use_s,
                                       in1=ohd[:], op0=Alu.mult, op1=Alu.add)

        # broadcast onehot to 128 partitions via PE; select chosen tokens
        ohb_ps = ps.tile([128, K], f32)
        nc.tensor.matmul(out=ohb_ps[:], lhsT=ones_row[:], rhs=oh_row[:],
                         start=True, stop=True)
        prod2 = sb.tile([128, K], f32)
        nc.vector.tensor_tensor(out=prod2[:], in0=ktokt_f[:], in1=ohb_ps[:], op=Alu.mult)
        tok_col = sb.tile([128, 1], f32)
        nc.vector.tensor_reduce(out=tok_col[:], in_=prod2[:], axis=AX, op=Alu.add)
        tok_i32 = sb.tile([128, 1], i32)
        nc.vector.tensor_copy(out=tok_i32[:], in_=tok_col[:])

        # ---- final gather of the chosen sentence (1 idx per partition)
        cs_g = sb.tile([128, D], f32)
        nc.gpsimd.indirect_dma_start(
            out=cs_g[:], out_offset=None, in_=emb[:, :],
            in_offset=bass.IndirectOffsetOnAxis(ap=tok_i32[:, :], axis=0))

        # ---- outputs
        nc.sync.dma_start(out=full_enc[0:Tk, :], in_=cs_g[:])
        nc.sync.dma_start(out=full_enc[Tk:Tk + 128, :], in_=ctx_g[:, 0:D])
        nc.sync.dma_start(out=full_enc[Tk + 128:Tk + 256, :], in_=ctx_g[:, D:2 * D])

        # cs mask: transpose tok_col to rows {0,32,64,96} then strided-AP DMA
        sq4 = sb.tile([128, 32], f32)
        nc.vector.memset(sq4[:], 0.0)
        nc.vector.tensor_copy(out=sq4[:, 0:1], in_=tok_col[:])
        sq4T = sb.tile([128, 32], f32)
        nc.vector.transpose(out=sq4T[:], in_=sq4[:])
        m4 = sb.tile([128, 32], u8)
        nc.vector.tensor_scalar(out=m4[:], in0=sq4T[:], scalar1=0, scalar2=None,
                                op0=Alu.not_equal)
        for b in range(4):
            nc.sync.dma_start(out=full_mask[32 * b:32 * (b + 1)],
                              in_=m4[32 * b:32 * b + 1, 0:32])

        cmask_u8 = sb.tile([1, Tc], u8)
        nc.vector.tensor_scalar(out=cmask_u8[:], in0=stokrow_sb[:], scalar1=0,
                                scalar2=None, op0=Alu.not_equal)
        nc.sync.dma_start(out=full_mask[Tk:Tk + Tc], in_=cmask_u8[:])

    nc.compile()
    return nc


def _get_compiled():
    global _compiled
    if _compiled is None:
        _compiled = _build()
    return _compiled


def _wrap16(flat):
    """dma_gather int16 index layout: idx[p, s] = flat[s*16 + p], tiled to 128 rows."""
    w = flat.astype(np.int16).reshape(-1, 16).T  # [16, n/16]
    return np.tile(w, (8, 1)).copy()


def kernel(src_tokens, know_tokens, ck_mask, cs_ids, use_cs_ids, emb):
    from concourse.bass_utils import run_bass_kernel_spmd

    src_tokens = np.asarray(src_tokens)
    know_tokens = np.asarray(know_tokens)
    ck_mask = np.asarray(ck_mask)
    cs_ids = np.asarray(cs_ids)
    use_flag = float(np.asarray(use_cs_ids))
    emb = np.ascontiguousarray(np.asarray(emb, dtype=np.float32))

    nc = _get_compiled()

    in_maps = []
    for n in range(N):
        kt = know_tokens[n]                       # [K, Tk]
        st = src_tokens[n]                        # [Tc]
        oh = np.zeros(K, dtype=np.float32)
        oh[int(cs_ids[n])] = 1.0
        rconsts = np.zeros((1, 66), dtype=np.float32)
        rconsts[0, 0:K] = 64.0 - np.arange(K, dtype=np.float32)
        rconsts[0, K:2 * K] = oh
        rconsts[0, 64] = use_flag
        cconsts = np.zeros((K, 2), dtype=np.float32)
        cconsts[:, 0] = ck_mask[n].astype(np.float32)
        cconsts[:, 1] = np.where(ck_mask[n], 0.0, NEGINF).astype(np.float32)
        in_maps.append({
            "emb": emb,
            "kidx16": _wrap16(kt.reshape(-1)),
            "sidx16": _wrap16(st),
            "ktokt": np.ascontiguousarray(kt.T.astype(np.int32)),
            "stokt": np.ascontiguousarray(st.reshape(2, 128).T.astype(np.int32)),
            "stokrow": st.astype(np.float32).reshape(1, Tc),
            "cconsts": cconsts,
            "rconsts": rconsts,
        })

    res = run_bass_kernel_spmd(nc, in_maps, core_ids=list(range(N)))
    global last_results
    last_results = res
    full_enc = np.stack([r["full_enc"] for r in res.results])
    full_mask = np.stack([r["full_mask"] for r in res.results]).astype(bool)
    ck_attn = np.stack([r["ck_attn"] for r in res.results])
    return full_enc, full_mask, ck_attn
